# revision 17
# baseline (speedup 1.0000x reference)
"""Trainium2 Bass kernel for nn_ExportGatedDeltaNet (gated linear attention
with depthwise conv, chunked recurrence).

Self-contained: hardcodes shapes/sharding. Sharding: 8-way tensor-parallel
over heads (each core owns 4 of the 32 value heads / 2 of the 16 key heads);
both batch elements are processed sequentially on every core. Each core
computes a full [B, T, C] partial of the output projection over its head
slice; the host sums the 8 partials.

v3: PE-warmth restructure - the projection matmul stream of block j+1 and
the output-projection stream of block j-1 are wedged between every
dependent step of block j's recurrence so the tensor engine never idles
long enough for the HAM clock gate to re-throttle. Activation-table
discipline: the sigmoid set is eliminated (log-beta and softplus are
computed from Exp/Ln which share one table set with the chunk/norm math),
leaving 2 table loads per block (Silu <-> Ln/Exp). The depthwise conv is
split across DVE and GpSimd; PSUM->SBUF stores are spread over Scalar /
Vector / GpSimd.
"""

import numpy as np
import ml_dtypes

import concourse.bass as bass
import concourse.tile as tile
from concourse import mybir
from concourse.vector_clock import ScopedClock, VectorClock
from concourse.bass_utils import run_bass_kernel_spmd

F32 = mybir.dt.float32
F32R = mybir.dt.float32r
BF16 = mybir.dt.bfloat16
AF = mybir.ActivationFunctionType
OP = mybir.AluOpType
BF16_NP = ml_dtypes.bfloat16

NK, NV, DK, DV, KCONV, C = 16, 32, 128, 128, 4, 2048
KEY = NK * DK            # 2048
B, T = 2, 2048
L = 128                  # recurrence chunk length
TB = 512                 # t-block
NTB = T // TB            # 4
NCH = TB // L            # chunks per t-block
NCORES = 8
EPS = 1e-6

# per-core head slice
VH = NV // NCORES        # 4 value heads
KH = NK // NCORES        # 2 key heads
QCH = KH * DK            # 256
VCH = VH * DV            # 512
ZCH = VH * DV            # 512
CONVCH = 2 * QCH + VCH   # 1024 channels through the conv
TOTCH = CONVCH + ZCH + 32 + VH  # 1572: ..., b(4), pad(28), a(4)
N_CT = C // 128          # 16 contraction tiles
NBLK = B * NTB           # 8 pipelined blocks


def _walrus_safe_drain(self, tick_clock, wait_clock):
    # The container's walrus rejects >1 sync-wait on CTRL-class instructions;
    # split the final drain's waits across single-wait nops.
    vals = eval(repr(tick_clock.global_clock).replace("VectorClock", ""))
    for j, v in enumerate(vals):
        if not v:
            continue
        masked = [0] * len(vals)
        masked[j] = v
        nop_inst = self.nc.sync.nop(nofuse=True)
        wait_clock.add_sem_waits(
            nop_inst.ins, ScopedClock({None: VectorClock(masked)})
        )
    self.nc.sync.drain()
    self.nc.all_engine_barrier()
    popped = self.nc._tile_sem_poison_stack.pop()
    assert popped is self._sem_poison
    self.nc.clear_and_free_semaphores(list(self.sems.allocated().values()))
    self.nc.all_engine_barrier()


tile.TileContext._drain_and_barrier = _walrus_safe_drain


# The container's walrus rejects >1 sync-wait on any instruction. Tile's
# semaphore pass emits multi-wait instructions, so split them at the BIR-JSON
# level: hoist all but one wait onto NoOps (same engine) inserted just before.
_orig_to_json_bytes = bass.Bass.to_json_bytes
_WSPLIT = [0]


def _split_multi_waits(self, *args, **kwargs):
    import json
    raw = _orig_to_json_bytes(self, *args, **kwargs)
    m = json.loads(raw)
    changed = False
    for f in m["functions"]:
        for bb in f["blocks"]:
            out_insts = []
            for inst in bb["instructions"]:
                si = inst.get("sync_info")
                waits = (si or {}).get("on_wait") or []
                if len(waits) > 1:
                    changed = True
                    for w in waits[:-1]:
                        _WSPLIT[0] += 1
                        out_insts.append({
                            "debug": inst.get("debug"),
                            "engine": inst["engine"],
                            "ins": [], "outs": [],
                            "name": f"I-wsplit-{_WSPLIT[0]}",
                            "opcode": "NoOp",
                            "sync_info": {"on_update": [], "on_wait": [w]},
                        })
                    si["on_wait"] = [waits[-1]]
                out_insts.append(inst)
            bb["instructions"] = out_insts
    if not changed:
        return raw
    return json.dumps(m).encode()


bass.Bass.to_json_bytes = _split_multi_waits

# HWDGE DMAs execute on DMA-queue timelines, where a hoisted same-engine NoOp
# wait does not gate them. Route static DMAs through the SP sequencer instead
# so program order (and the NoOp wait splitting) applies to them too.
import concourse.bass_utils as _bu

_orig_run_command = _bu.run_command


def _patched_run_command(argv, **kwargs):
    argv = [a.replace("--assign-static-dmas-to-sp=false",
                      "--assign-static-dmas-to-sp=true") for a in argv]
    return _orig_run_command(argv, **kwargs)


_bu.run_command = _patched_run_command


def build_kernel():
    nc = bass.Bass(num_swdge_queues=4)

    xt = nc.dram_tensor("xt", [B, C, T], BF16, kind="ExternalInput")
    wt = nc.dram_tensor("wt", [C, TOTCH], BF16, kind="ExternalInput")
    wout = nc.dram_tensor("wout", [VCH, C], BF16, kind="ExternalInput")
    convw = nc.dram_tensor("convw", [128, CONVCH // 128, KCONV], F32,
                           kind="ExternalInput")
    halo = nc.dram_tensor("halo", [B, 128, CONVCH // 128, KCONV - 1], BF16,
                          kind="ExternalInput")
    s0 = nc.dram_tensor("s0", [B, VH, DK, DV], F32, kind="ExternalInput")
    dtb = nc.dram_tensor("dtb", [VH, 1], F32, kind="ExternalInput")
    nega = nc.dram_tensor("nega", [VH, 1], F32, kind="ExternalInput")
    normw = nc.dram_tensor("normw", [128, 1], F32, kind="ExternalInput")
    out = nc.dram_tensor("out", [B, T, C], BF16, kind="ExternalOutput")

    # constants embedded in the NEFF
    ut_np = np.triu(np.ones((L, L), np.float32))              # [u,t]: u<=t
    UT = nc.inline_tensor(ut_np.astype(BF16_NP), name="UT")
    STA = nc.inline_tensor((1.0 - ut_np).astype(BF16_NP), name="STA")  # u>s
    ONESM = nc.inline_tensor(np.ones((L, L), BF16_NP), name="ONESM")
    NEGM = nc.inline_tensor(
        np.where(ut_np > 0, 0.0, -1e30).astype(np.float32), name="NEGM")
    IDENT = nc.inline_tensor(np.eye(8, dtype=np.float32), name="IDENT")
    ONES_COL = nc.inline_tensor(np.ones((128, 1), BF16_NP), name="ONES_COL")
    ONES_ROW = nc.inline_tensor(np.ones((1, 128), np.float32), name="ONES_ROW")
    EPS_T = nc.inline_tensor(np.full((1, 1), EPS, np.float32), name="EPS_T")
    ONE_VH = nc.inline_tensor(np.ones((VH, 1), np.float32), name="ONE_VH")

    n_convt = CONVCH // 128   # 8 conv channel tiles
    n_zt = ZCH // 128         # 4
    n_wt = TOTCH // 128       # 12 full tiles + 8 extra cols in partial tile

    from contextlib import ExitStack
    with nc.allow_low_precision(reason="bf16/f32r compute by design"), \
         tile.TileContext(nc) as tc, ExitStack() as stack:
        consts = stack.enter_context(tc.tile_pool(name="consts", bufs=1))
        wpool = stack.enter_context(tc.tile_pool(name="wpool", bufs=1))
        xpool = stack.enter_context(tc.tile_pool(name="xpool", bufs=2))
        rawp = stack.enter_context(tc.tile_pool(name="rawp", bufs=2))
        sbig = stack.enter_context(tc.tile_pool(name="sbig", bufs=2))
        stiny = stack.enter_context(tc.tile_pool(name="stiny", bufs=2))
        stiny1 = stack.enter_context(tc.tile_pool(name="stiny1", bufs=1))
        sbig1 = stack.enter_context(tc.tile_pool(name="sbig1", bufs=1))
        stiny3 = stack.enter_context(tc.tile_pool(name="stiny3", bufs=3))
        tpool = stack.enter_context(tc.tile_pool(name="tpool", bufs=4))
        statep = stack.enter_context(tc.tile_pool(name="statep", bufs=1))
        pproj = stack.enter_context(tc.tile_pool(name="pproj", bufs=3, space="PSUM"))
        pddt = stack.enter_context(tc.tile_pool(name="pddt", bufs=2, space="PSUM"))
        pnorm = stack.enter_context(tc.tile_pool(name="pnorm", bufs=1, space="PSUM"))
        psmall = stack.enter_context(tc.tile_pool(name="psmall", bufs=2, space="PSUM"))

        # load constants to SBUF
        ut_sb = consts.tile([L, L], BF16)
        nc.gpsimd.dma_start(ut_sb[:], UT[:])
        sta_sb = consts.tile([L, L], BF16)
        nc.gpsimd.dma_start(sta_sb[:], STA[:])
        onesm_sb = consts.tile([L, L], BF16)
        nc.gpsimd.dma_start(onesm_sb[:], ONESM[:])
        negm_sb = consts.tile([L, L], F32)
        nc.gpsimd.dma_start(negm_sb[:], NEGM[:])
        ident_sb = consts.tile([8, 8], F32)
        nc.gpsimd.dma_start(ident_sb[:], IDENT[:])
        onescol_sb = consts.tile([128, 1], BF16)
        nc.gpsimd.dma_start(onescol_sb[:], ONES_COL[:])
        onesrow_sb = consts.tile([1, 128], F32R)
        nc.gpsimd.dma_start(onesrow_sb[:], ONES_ROW[:].bitcast(F32R))
        convw_sb = consts.tile([128, n_convt, KCONV], F32)
        nc.gpsimd.dma_start(convw_sb[:], convw[:])
        dtb_sb = consts.tile([VH, 1], F32)
        nc.gpsimd.dma_start(dtb_sb[:], dtb[:])
        nega_sb = consts.tile([VH, 1], F32)
        nc.gpsimd.dma_start(nega_sb[:], nega[:])
        normw_sb = consts.tile([128, 1], F32)
        nc.gpsimd.dma_start(normw_sb[:], normw[:])
        eps_sb = consts.tile([1, 1], F32)
        nc.gpsimd.dma_start(eps_sb[:], EPS_T[:])
        onevh_sb = consts.tile([VH, 1], F32)
        nc.gpsimd.dma_start(onevh_sb[:], ONE_VH[:])

        # resident weights
        wt_sb = wpool.tile([128, N_CT, TOTCH], BF16)
        nc.gpsimd.dma_start(wt_sb[:], wt.rearrange("(ko p) f -> p ko f", p=128))
        wout_sb = wpool.tile([128, VH, C], BF16)
        nc.gpsimd.dma_start(wout_sb[:], wout.rearrange("(vo p) f -> p vo f", p=128))

        blocks = [(b, tb) for b in range(B) for tb in range(NTB)]
        xts = {}       # j -> xt tile
        raws = {}      # j -> conv-input raw tile
        zraws = {}     # j -> z pre-activation tile
        accs = {}      # j -> conv accumulator tile [128, n_convt, TB]
        zs = {}        # j -> silu(z) tile
        fqs = {}       # j -> silu'd conv q/k tiles (list)
        vs = {}        # j -> v tile
        qns = {}       # j -> normalized q
        kns = {}       # j -> normalized k
        gsps = {}      # j -> g / softplus(-b) tile
        ogs = {}       # j -> recurrence output
        ktTs = {}      # (j, c) -> transposed k
        vTs = {}       # (j, c) -> transposed v
        Ss = {}        # b -> state tile

        def emit_load(j):
            if j >= NBLK:
                return
            b, tb = blocks[j]
            tsl = slice(tb * TB, (tb + 1) * TB)
            xt_sb = xpool.tile([128, N_CT, TB], BF16, tag="xt")
            nc.gpsimd.dma_start(
                xt_sb[:],
                xt[b].rearrange("(ko p) t -> p ko t", p=128)[:, :, tsl])
            xts[j] = xt_sb

        # ---------------- projection pieces for block j ----------------
        def emit_zproj(j, zi, eng):
            """One z tile: 16 accum MMs + copy to zraw on engine `eng`."""
            if j >= NBLK:
                return
            if zi == 0:
                zraw = sbig.tile([128, n_zt, TB], BF16, tag="zraw")
                zraws[j] = zraw
            xt_sb = xts[j]
            cht = n_convt + zi
            ps = pproj.tile([128, TB], F32, tag="proj")
            wcols = slice(cht * 128, (cht + 1) * 128)
            for ct in range(N_CT):
                nc.tensor.matmul(ps[:], wt_sb[:, ct, wcols], xt_sb[:, ct, :],
                                 start=(ct == 0), stop=(ct == N_CT - 1))
            eng(zraws[j][:, zi, :], ps[:])

        def emit_baproj(j):
            """b/a tile MMs + g / softplus(-b) computed straight from PSUM
            with Exp/Ln (no sigmoid table)."""
            if j >= NBLK:
                return
            xt_sb = xts[j]
            ps = pproj.tile([128, TB], F32, tag="proj")
            rows = slice(0, 32 + VH)
            wcols = slice(n_wt * 128, TOTCH)
            for ct in range(N_CT):
                nc.tensor.matmul(ps[rows, :], wt_sb[:, ct, wcols],
                                 xt_sb[:, ct, :],
                                 start=(ct == 0), stop=(ct == N_CT - 1))
            gsp = stiny.tile([VH, 2, TB], F32, tag="gsp")
            gsps[j] = gsp
            # softplus(-b) = -log(sigmoid(b)) = ln(1 + exp(-b))
            t1 = stiny1.tile([VH, TB], F32, tag="t1")
            nc.scalar.activation(t1[:], ps[0:VH, :], AF.Exp, scale=-1.0)
            # softplus(a + dtb) = ln(1 + exp(a + dtb))
            t2 = stiny1.tile([VH, TB], F32, tag="t2")
            nc.scalar.activation(t2[:], ps[32:32 + VH, :], AF.Exp,
                                 bias=dtb_sb[:])
            nc.scalar.activation(gsp[:, 1, :], t1[:], AF.Ln, bias=onevh_sb[:])
            sp = stiny1.tile([VH, TB], F32, tag="t1")
            nc.scalar.activation(sp[:], t2[:], AF.Ln, bias=onevh_sb[:])
            # g = -exp(A) * softplus(a + dtb)
            nc.vector.tensor_scalar(gsp[:, 0, :], sp[:], nega_sb[:],
                                    None, OP.mult)

        def emit_convproj(j, cht, eng):
            """One conv-channel tile: 16 accum MMs + copy into raw."""
            if j >= NBLK:
                return
            b, tb = blocks[j]
            if cht == 0:
                raw = rawp.tile([128, n_convt, TB + KCONV - 1], BF16, tag="raw")
                raws[j] = raw
                if tb == 0:
                    nc.gpsimd.dma_start(raw[:, :, 0:3], halo[b])
                else:
                    nc.scalar.copy(raw[:, :, 0:3], raws[j - 1][:, :, TB:TB + 3])
                acc = sbig.tile([128, n_convt, TB], BF16, tag="acc", bufs=1)
                accs[j] = acc
            xt_sb = xts[j]
            ps = pproj.tile([128, TB], F32, tag="proj")
            wcols = slice(cht * 128, (cht + 1) * 128)
            for ct in range(N_CT):
                nc.tensor.matmul(ps[:], wt_sb[:, ct, wcols], xt_sb[:, ct, :],
                                 start=(ct == 0), stop=(ct == N_CT - 1))
            eng(raws[j][:, cht, 3:TB + 3], ps[:])

        def emit_conv(j, cht, on_gpsimd=False):
            """4-tap depthwise conv for one channel tile (DVE or GpSimd)."""
            if j >= NBLK:
                return
            raw = raws[j]
            acc = accs[j]
            eng = nc.vector
            eng.tensor_scalar(acc[:, cht, :], raw[:, cht, 0:TB],
                              convw_sb[:, cht, 0:1], None, OP.mult)
            for jj in range(1, KCONV):
                eng.scalar_tensor_tensor(
                    acc[:, cht, :], raw[:, cht, jj:TB + jj],
                    convw_sb[:, cht, jj:jj + 1], acc[:, cht, :],
                    OP.mult, OP.add)

        # ---------------- silu batch for block j ----------------
        def emit_silu(j):
            acc = accs[j]
            z_sb = sbig1.tile([128, n_zt, TB], BF16, tag="z")
            zs[j] = z_sb
            v_sb = sbig1.tile([128, VH, TB], BF16, tag="v")
            vs[j] = v_sb
            fq = []
            for cht in range(2 * KH):
                f = stiny1.tile([128, TB], BF16, tag=f"qkf{cht}")
                nc.scalar.activation(f[:], acc[:, cht, :], AF.Silu)
                fq.append(f)
            fqs[j] = fq
            for cht in range(2 * KH, n_convt):
                nc.scalar.activation(v_sb[:, cht - 2 * KH, :],
                                     acc[:, cht, :], AF.Silu)
            for zi in range(n_zt):
                nc.scalar.activation(z_sb[:, zi, :], zraws[j][:, zi, :],
                                     AF.Silu)
            for c in range(NCH):
                vT = tpool.tile([128, VH, L], BF16, tag="vT")
                for h in range(VH):
                    nc.sync.dma_start_transpose(
                        vT[:, h, :], v_sb[:, h, c * L:c * L + L])
                vTs[(j, c)] = vT

        # ---------------- qk norm for block j (batched, Ln/Exp set) -----
        def emit_norm_pair(srcs, dsts, wedge=None, eps=False):
            """L2-norm two [128,TB] tiles: sq (DVE), ssq (PE), Ln+Exp (ACT),
            rb (PE), mul (DVE) -> dsts. Pairwise keeps pool rotation
            monotone (no FIFO inversion deadlocks). `wedge` emits filler PE
            work between ssq and rb so the PE never stalls on the Ln/Exp."""
            sqs = []
            for f in srcs:
                sq = stiny.tile([128, TB], BF16, tag="sq")
                fa = f[:] if hasattr(f, 'tag') else f
                nc.vector.tensor_tensor(sq[:], fa, fa, OP.mult)
                sqs.append(sq)
            ssqs = []
            for sq in sqs:
                ssq = pnorm.tile([1, TB], F32, tag="nrm")
                nc.tensor.matmul(ssq[:], onescol_sb[:], sq[:],
                                 start=True, stop=True)
                ssqs.append(ssq)
            rinvs = []
            for ssq in ssqs:
                lssq = stiny1.tile([1, TB], F32, tag="lssq", bufs=2)
                if eps:
                    nc.scalar.activation(lssq[:], ssq[:], AF.Ln,
                                         bias=eps_sb[:], scale=1.0 / DV)
                else:
                    nc.scalar.activation(lssq[:], ssq[:], AF.Ln)
                rinv = stiny1.tile([1, TB], F32R, tag="sroot", bufs=2)
                nc.scalar.activation(rinv[:], lssq[:], AF.Exp, scale=-0.5)
                rinvs.append(rinv)
            if wedge is not None:
                wedge()
            for (f, dst, op), rinv in zip(dsts, rinvs):
                rb = pnorm.tile([128, TB], F32, tag="nrm")
                nc.tensor.matmul(rb[:], onesrow_sb[:], rinv[:],
                                 start=True, stop=True)
                if op is None:
                    nc.vector.tensor_tensor(dst, f[:], rb[:], OP.mult)
                else:
                    nc.vector.scalar_tensor_tensor(dst, f[:], normw_sb[:],
                                                   rb[:], OP.mult, OP.mult)

        def emit_qk_transposes(j):
            kn_sb = kns[j]
            for c in range(NCH):
                ktT = tpool.tile([128, KH, L], BF16, tag="ktT")
                for kh in range(KH):
                    nc.sync.dma_start_transpose(
                        ktT[:, kh, :], kn_sb[:, kh, c * L:c * L + L])
                ktTs[(j, c)] = ktT

        # ---------------- one recurrence chunk of block j ----------------
        def emit_chunk(j, c):
            b, tb = blocks[j]
            S = Ss[b]
            gsp = gsps[j]
            qn_sb, kn_sb = qns[j], kns[j]
            z_sb = zs[j]
            og_sb = ogs[j]
            t0 = c * L
            ktT, vT = ktTs[(j, c)], vTs[(j, c)]

            # transpose gsp chunk -> gspT [128, 2VH]
            tps = psmall.tile([128, L], F32, tag="mm128")
            nc.tensor.transpose(tps[:, 0:VH],
                                gsp[:, 0, t0:t0 + L], ident_sb[0:VH, 0:VH])
            nc.tensor.transpose(tps[:, VH:2 * VH],
                                gsp[:, 1, t0:t0 + L], ident_sb[0:VH, 0:VH])
            gspT = stiny3.tile([128, 2 * VH], F32, tag="gspT")
            nc.vector.tensor_copy(gspT[:], tps[:, 0:2 * VH])

            Ball = stiny.tile([128, VH, L], BF16, tag="Ball")
            for h in range(VH):
                nc.vector.tensor_scalar(
                    Ball[:, h, :], ut_sb[:], gspT[:, h:h + 1],
                    None, OP.mult)
            Dps = pddt.tile([128, VH * L], F32, tag="ddt")
            nc.tensor.matmul(Dps[:], sta_sb[:],
                             Ball[:].rearrange("p a b -> p (a b)"),
                             start=True, stop=True)
            dtps = pddt.tile([128, VH * L], F32, tag="ddt")
            nc.tensor.matmul(dtps[:], onesm_sb[:],
                             Ball[:].rearrange("p a b -> p (a b)"),
                             start=True, stop=True)
            ebr = stiny.tile([128, VH, L], F32, tag="ebr")
            nc.scalar.activation(
                ebr[:].rearrange("p a b -> p (a b)"), dtps[:], AF.Exp)
            Eall = stiny.tile([128, VH, L], F32, tag="Eall")
            for h in range(VH):
                # gspT[:, VH+h] holds softplus(-b) = -ln(beta): subtract it
                nc.vector.scalar_tensor_tensor(
                    Eall[:, h, :], Dps[:, h * L:(h + 1) * L],
                    gspT[:, VH + h:VH + h + 1], negm_sb[:],
                    OP.subtract, OP.add)
            Decay = stiny1.tile([128, VH, L], F32, tag="Decay")
            nc.scalar.activation(
                Decay[:].rearrange("p a b -> p (a b)"),
                Eall[:].rearrange("p a b -> p (a b)"), AF.Exp)

            Pps = []
            for kh in range(KH):
                pp = psmall.tile([128, L], F32, tag="mm128")
                nc.tensor.matmul(pp[:], kn_sb[:, kh, t0:t0 + L],
                                 qn_sb[:, kh, t0:t0 + L],
                                 start=True, stop=True)
                Pps.append(pp)

            for h in range(VH):
                kh = h // 2
                qh = stiny3.tile([128, L], F32R, tag="qh")
                nc.vector.tensor_tensor(
                    qh[:], qn_sb[:, kh, t0:t0 + L], ebr[:, h, :],
                    OP.mult)
                PT = stiny3.tile([128, L], BF16, tag="PT")
                nc.vector.tensor_tensor(PT[:], Pps[kh][:],
                                        Decay[:, h, :], OP.mult)
                ops = psmall.tile([128, L], F32, tag="mm128")
                nc.tensor.matmul(ops[:], S[:, h, :], qh[:],
                                 start=True, stop=False)
                nc.tensor.matmul(ops[:], vT[:, h, :], PT[:],
                                 start=False, stop=True)
                nc.vector.tensor_tensor(og_sb[:, h, t0:t0 + L],
                                        ops[:], z_sb[:, h, t0:t0 + L],
                                        OP.mult)
                kt2 = stiny3.tile([128, L], BF16, tag="kt2")
                nc.vector.tensor_scalar(
                    kt2[:], ktT[:, kh, :], Decay[:, h, L - 1:L],
                    None, OP.mult)
                sps = psmall.tile([128, L], F32, tag="mm128")
                nc.tensor.matmul(sps[:], kt2[:], vT[:, h, :],
                                 start=True, stop=True)
                nc.vector.scalar_tensor_tensor(
                    S[:, h, :], S[:, h, :], ebr[:, h, L - 1:L],
                    sps[:], OP.mult, OP.add)

        # ---------------- gated RMSNorm (batched) + out-proj ----------
        def emit_rms_pair(j, ogn_sb, hs, wedge=None):
            og_sb = ogs[j]
            emit_norm_pair([og_sb[:, h, :] for h in hs],
                           [(og_sb[:, h, :], ogn_sb[:, h, :], 'stt')
                            for h in hs], wedge=wedge, eps=True)

        def emit_outproj_tile(j, ogn_sb, c, co, eng):
            """One [L,512] out-proj psum tile + store."""
            b, tb = blocks[j]
            rows = slice(tb * TB + c * L, tb * TB + (c + 1) * L)
            ops2 = pproj.tile([128, 512], F32, tag="proj")
            for h in range(VH):
                nc.tensor.matmul(
                    ops2[:], ogn_sb[:, h, c * L:(c + 1) * L],
                    wout_sb[:, h, co * 512:(co + 1) * 512],
                    start=(h == 0), stop=(h == VH - 1))
            ost = stiny1.tile([128, 512], BF16, tag=f"ost{(c * 4 + co) % 3}")
            eng(ost[:], ops2[:])
            nc.gpsimd.dma_start(out[b, rows, co * 512:(co + 1) * 512], ost[:])

        # engines for copies, round-robin
        def cp_scalar(dst, src):
            nc.scalar.copy(dst, src)

        def cp_vector(dst, src):
            nc.vector.tensor_copy(dst, src)

        def cp_gpsimd(dst, src):
            nc.gpsimd.tensor_copy(dst, src)

        # ================= pipelined emission =================
        # prologue: block 0's projections + conv, block 1's loads
        emit_load(0)
        emit_load(1)
        for zi in range(n_zt):
            emit_zproj(0, zi, cp_vector)
        emit_baproj(0)
        for cht in range(n_convt):
            emit_convproj(0, cht, cp_scalar)
        for cht in range(n_convt):
            emit_conv(0, cht)

        ogn_prev = None
        for j in range(NBLK):
            b, tb = blocks[j]
            emit_load(j + 2)

            # ---- phase A: rms(j-1) + outproj(j-1) + z/ba proj(j+1) ----
            emit_zproj(j + 1, 0, cp_vector)
            if j >= 1:
                ogn_prev = sbig1.tile([128, VH, TB], BF16, tag="ogn")
                emit_rms_pair(j - 1, ogn_prev, [0, 1],
                              wedge=lambda: emit_zproj(j + 1, 1, cp_vector))
                emit_rms_pair(j - 1, ogn_prev, [2, 3],
                              wedge=lambda: emit_zproj(j + 1, 2, cp_vector))
            else:
                emit_zproj(j + 1, 1, cp_vector)
                emit_zproj(j + 1, 2, cp_vector)
            if j >= 1:
                for c in range(NCH):
                    for co in range(2):
                        emit_outproj_tile(j - 1, ogn_prev, c, co,
                                          cp_scalar if co == 0 else cp_vector)
            emit_zproj(j + 1, 3, cp_vector)
            emit_baproj(j + 1)
            if j >= 1:
                for c in range(NCH):
                    emit_outproj_tile(j - 1, ogn_prev, c, 2,
                                      cp_scalar if c < 2 else cp_vector)
            for cht in range(4):
                emit_convproj(j + 1, cht, cp_scalar)
                emit_conv(j + 1, cht)

            # ---- phase B: silu batch for block j ----
            emit_silu(j)

            # ---- phase C: qknorm(j) + chunks(j), wedged with proj(j+1) --
            fq = fqs[j]
            qn_sb = sbig1.tile([128, KH, TB], BF16, tag="qn")
            kn_sb = sbig1.tile([128, KH, TB], BF16, tag="kn")
            qns[j], kns[j] = qn_sb, kn_sb
            emit_norm_pair(fq[0:2],
                           [(fq[0], qn_sb[:, 0, :], None),
                            (fq[1], qn_sb[:, 1, :], None)],
                           wedge=lambda: (emit_convproj(j + 1, 4, cp_vector),
                                          emit_conv(j + 1, 4)))
            emit_norm_pair(fq[2:4],
                           [(fq[2], kn_sb[:, 0, :], None),
                            (fq[3], kn_sb[:, 1, :], None)],
                           wedge=lambda: (emit_convproj(j + 1, 5, cp_vector),
                                          emit_conv(j + 1, 5)))
            emit_qk_transposes(j)

            if tb == 0:
                S = statep.tile([128, VH, DV], F32R, tag=f"S{b}")
                nc.gpsimd.dma_start(
                    S[:], s0[b].rearrange("h d v -> d h v").bitcast(F32R))
                Ss[b] = S
            og_sb = sbig.tile([128, VH, TB], BF16, tag="og", bufs=1)
            ogs[j] = og_sb

            for c in range(NCH):
                if c == 1:
                    emit_convproj(j + 1, 6, cp_vector)
                    emit_conv(j + 1, 6)
                if c == 2:
                    emit_convproj(j + 1, 7, cp_vector)
                    emit_conv(j + 1, 7)
                if c >= 1 and j >= 1:
                    # out-proj co=3 tiles as PE fillers inside the chunk loop
                    emit_outproj_tile(j - 1, ogn_prev, c - 1, 3, cp_vector)
                emit_chunk(j, c)
            if j >= 1:
                emit_outproj_tile(j - 1, ogn_prev, 3, 3, cp_vector)

        # epilogue
        ogn_last = sbig1.tile([128, VH, TB], BF16, tag="ogn")
        emit_rms_pair(NBLK - 1, ogn_last, [0, 1])
        emit_rms_pair(NBLK - 1, ogn_last, [2, 3])
        for c in range(NCH):
            for co in range(4):
                emit_outproj_tile(NBLK - 1, ogn_last, c, co,
                                  (cp_scalar, cp_vector, cp_scalar,
                                   cp_vector)[co])

    return nc


_NC_CACHE = None
LAST_RESULT = None


def kernel(**inputs):
    global _NC_CACHE, LAST_RESULT
    x = np.asarray(inputs["x"], np.float32)
    input_pos = np.asarray(inputs["input_pos"])
    W_qkv = np.asarray(inputs["W_qkv"], np.float32)
    W_z = np.asarray(inputs["W_z"], np.float32)
    W_b = np.asarray(inputs["W_b"], np.float32)
    W_a = np.asarray(inputs["W_a"], np.float32)
    conv_w = np.asarray(inputs["conv_w"], np.float32)[:, 0, :]
    dt_bias = np.asarray(inputs["dt_bias"], np.float32)
    A_log = np.asarray(inputs["A_log"], np.float32)
    norm_w = np.asarray(inputs["norm_w"], np.float32)
    W_out = np.asarray(inputs["W_out"], np.float32)
    conv_state = np.asarray(inputs["conv_state"], np.float32)
    rec_state = np.asarray(inputs["recurrent_state"], np.float32)

    keep = 0.0 if int(input_pos[0]) == 0 else 1.0
    conv_state = conv_state * keep
    rec_state = rec_state * keep

    xt_host = np.ascontiguousarray(x.transpose(0, 2, 1)).astype(BF16_NP)

    in_maps = []
    for core in range(NCORES):
        vh = slice(VH * core, VH * (core + 1))
        qrows = slice(QCH * core, QCH * (core + 1))
        krows = slice(KEY + QCH * core, KEY + QCH * (core + 1))
        vrows = slice(2 * KEY + VCH * core, 2 * KEY + VCH * (core + 1))
        zrows = slice(ZCH * core, ZCH * (core + 1))

        Wt = np.concatenate(
            [W_qkv[qrows], W_qkv[krows], W_qkv[vrows], W_z[zrows],
             W_b[vh], np.zeros((32 - VH, C), np.float32),
             W_a[vh]], axis=0)                    # [TOTCH, C]
        wt_host = np.ascontiguousarray(Wt.T).astype(BF16_NP)      # [C, TOTCH]
        wout_host = np.ascontiguousarray(
            W_out[:, VCH * core:VCH * (core + 1)].T).astype(BF16_NP)

        cw = np.concatenate([conv_w[qrows], conv_w[krows], conv_w[vrows]], 0)
        convw_host = np.ascontiguousarray(
            cw.reshape(CONVCH // 128, 128, KCONV).transpose(1, 0, 2))

        cs = np.concatenate([conv_state[:, qrows], conv_state[:, krows],
                             conv_state[:, vrows]], axis=1)       # [B,1024,4]
        halo_host = np.ascontiguousarray(
            cs[:, :, 1:4].reshape(B, CONVCH // 128, 128, 3)
            .transpose(0, 2, 1, 3)).astype(BF16_NP)

        s0_host = np.ascontiguousarray(rec_state[:, vh])          # [B,VH,DK,DV]
        dtb_host = np.ascontiguousarray(dt_bias[vh][:, None])
        nega_host = np.ascontiguousarray(-np.exp(A_log[vh])[:, None])
        normw_host = np.ascontiguousarray(norm_w[:, None])

        in_maps.append({
            "xt": xt_host, "wt": wt_host, "wout": wout_host,
            "convw": convw_host, "halo": halo_host, "s0": s0_host,
            "dtb": dtb_host, "nega": nega_host, "normw": normw_host,
        })

    if _NC_CACHE is None:
        _NC_CACHE = build_kernel()
    res = run_bass_kernel_spmd(_NC_CACHE, in_maps, core_ids=list(range(NCORES)))
    LAST_RESULT = res

    total = np.zeros((B, T, C), np.float32)
    for r in res.results:
        total += np.asarray(r["out"], dtype=np.float32)
    return total


# revision 18
# speedup vs baseline: 1.0300x; 1.0300x over previous
"""Trainium2 Bass kernel for nn_ExportGatedDeltaNet (gated linear attention
with depthwise conv, chunked recurrence).

Self-contained: hardcodes shapes/sharding. Sharding: 8-way tensor-parallel
over heads (each core owns 4 of the 32 value heads / 2 of the 16 key heads);
both batch elements are processed sequentially on every core. Each core
computes a full [B, T, C] partial of the output projection over its head
slice; the host sums the 8 partials.

v3: PE-warmth restructure - the projection matmul stream of block j+1 and
the output-projection stream of block j-1 are wedged between every
dependent step of block j's recurrence so the tensor engine never idles
long enough for the HAM clock gate to re-throttle. Activation-table
discipline: the sigmoid set is eliminated (log-beta and softplus are
computed from Exp/Ln which share one table set with the chunk/norm math),
leaving 2 table loads per block (Silu <-> Ln/Exp). The depthwise conv is
split across DVE and GpSimd; PSUM->SBUF stores are spread over Scalar /
Vector / GpSimd.
"""

import numpy as np
import ml_dtypes

import concourse.bass as bass
import concourse.tile as tile
from concourse import mybir
from concourse.vector_clock import ScopedClock, VectorClock
from concourse.bass_utils import run_bass_kernel_spmd

F32 = mybir.dt.float32
F32R = mybir.dt.float32r
BF16 = mybir.dt.bfloat16
AF = mybir.ActivationFunctionType
OP = mybir.AluOpType
BF16_NP = ml_dtypes.bfloat16

NK, NV, DK, DV, KCONV, C = 16, 32, 128, 128, 4, 2048
KEY = NK * DK            # 2048
B, T = 2, 2048
L = 128                  # recurrence chunk length
TB = 512                 # t-block
NTB = T // TB            # 4
NCH = TB // L            # chunks per t-block
NCORES = 8
EPS = 1e-6

# per-core head slice
VH = NV // NCORES        # 4 value heads
KH = NK // NCORES        # 2 key heads
QCH = KH * DK            # 256
VCH = VH * DV            # 512
ZCH = VH * DV            # 512
CONVCH = 2 * QCH + VCH   # 1024 channels through the conv
TOTCH = CONVCH + ZCH + 32 + VH  # 1572: ..., b(4), pad(28), a(4)
N_CT = C // 128          # 16 contraction tiles
NBLK = B * NTB           # 8 pipelined blocks


def _walrus_safe_drain(self, tick_clock, wait_clock):
    # The container's walrus rejects >1 sync-wait on CTRL-class instructions;
    # split the final drain's waits across single-wait nops.
    vals = eval(repr(tick_clock.global_clock).replace("VectorClock", ""))
    for j, v in enumerate(vals):
        if not v:
            continue
        masked = [0] * len(vals)
        masked[j] = v
        nop_inst = self.nc.sync.nop(nofuse=True)
        wait_clock.add_sem_waits(
            nop_inst.ins, ScopedClock({None: VectorClock(masked)})
        )
    self.nc.sync.drain()
    self.nc.all_engine_barrier()
    popped = self.nc._tile_sem_poison_stack.pop()
    assert popped is self._sem_poison
    self.nc.clear_and_free_semaphores(list(self.sems.allocated().values()))
    self.nc.all_engine_barrier()


tile.TileContext._drain_and_barrier = _walrus_safe_drain


# The container's walrus rejects >1 sync-wait on any instruction. Tile's
# semaphore pass emits multi-wait instructions, so split them at the BIR-JSON
# level: hoist all but one wait onto NoOps (same engine) inserted just before.
_orig_to_json_bytes = bass.Bass.to_json_bytes
_WSPLIT = [0]


def _split_multi_waits(self, *args, **kwargs):
    import json
    raw = _orig_to_json_bytes(self, *args, **kwargs)
    m = json.loads(raw)
    changed = False
    for f in m["functions"]:
        for bb in f["blocks"]:
            out_insts = []
            for inst in bb["instructions"]:
                si = inst.get("sync_info")
                waits = (si or {}).get("on_wait") or []
                if len(waits) > 1:
                    changed = True
                    for w in waits[:-1]:
                        _WSPLIT[0] += 1
                        out_insts.append({
                            "debug": inst.get("debug"),
                            "engine": inst["engine"],
                            "ins": [], "outs": [],
                            "name": f"I-wsplit-{_WSPLIT[0]}",
                            "opcode": "NoOp",
                            "sync_info": {"on_update": [], "on_wait": [w]},
                        })
                    si["on_wait"] = [waits[-1]]
                out_insts.append(inst)
            bb["instructions"] = out_insts
    if not changed:
        return raw
    return json.dumps(m).encode()


bass.Bass.to_json_bytes = _split_multi_waits

# HWDGE DMAs execute on DMA-queue timelines, where a hoisted same-engine NoOp
# wait does not gate them. Route static DMAs through the SP sequencer instead
# so program order (and the NoOp wait splitting) applies to them too.
import concourse.bass_utils as _bu

_orig_run_command = _bu.run_command


def _patched_run_command(argv, **kwargs):
    argv = [a.replace("--assign-static-dmas-to-sp=false",
                      "--assign-static-dmas-to-sp=true") for a in argv]
    return _orig_run_command(argv, **kwargs)


_bu.run_command = _patched_run_command


def build_kernel():
    nc = bass.Bass(num_swdge_queues=4)

    xt = nc.dram_tensor("xt", [B, C, T], BF16, kind="ExternalInput")
    wt = nc.dram_tensor("wt", [C, TOTCH], BF16, kind="ExternalInput")
    wout = nc.dram_tensor("wout", [VCH, C], BF16, kind="ExternalInput")
    convw = nc.dram_tensor("convw", [128, CONVCH // 128, KCONV], F32,
                           kind="ExternalInput")
    halo = nc.dram_tensor("halo", [B, 128, CONVCH // 128, KCONV - 1], BF16,
                          kind="ExternalInput")
    s0 = nc.dram_tensor("s0", [B, VH, DK, DV], F32, kind="ExternalInput")
    dtb = nc.dram_tensor("dtb", [VH, 1], F32, kind="ExternalInput")
    nega = nc.dram_tensor("nega", [VH, 1], F32, kind="ExternalInput")
    normw = nc.dram_tensor("normw", [128, 1], F32, kind="ExternalInput")
    out = nc.dram_tensor("out", [B, T, C], BF16, kind="ExternalOutput")

    # constants embedded in the NEFF
    ut_np = np.triu(np.ones((L, L), np.float32))              # [u,t]: u<=t
    UT = nc.inline_tensor(ut_np.astype(BF16_NP), name="UT")
    STA = nc.inline_tensor((1.0 - ut_np).astype(BF16_NP), name="STA")  # u>s
    ONESM = nc.inline_tensor(np.ones((L, L), BF16_NP), name="ONESM")
    NEGM = nc.inline_tensor(
        np.where(ut_np > 0, 0.0, -1e30).astype(np.float32), name="NEGM")
    IDENT = nc.inline_tensor(np.eye(8, dtype=np.float32), name="IDENT")
    ONES_COL = nc.inline_tensor(np.ones((128, 1), BF16_NP), name="ONES_COL")
    ONES_ROW = nc.inline_tensor(np.ones((1, 128), np.float32), name="ONES_ROW")
    EPS_T = nc.inline_tensor(np.full((1, 1), EPS, np.float32), name="EPS_T")
    ONE_VH = nc.inline_tensor(np.ones((VH, 1), np.float32), name="ONE_VH")

    n_convt = CONVCH // 128   # 8 conv channel tiles
    n_zt = ZCH // 128         # 4
    n_wt = TOTCH // 128       # 12 full tiles + 8 extra cols in partial tile

    from contextlib import ExitStack
    with nc.allow_low_precision(reason="bf16/f32r compute by design"), \
         tile.TileContext(nc) as tc, ExitStack() as stack:
        consts = stack.enter_context(tc.tile_pool(name="consts", bufs=1))
        wpool = stack.enter_context(tc.tile_pool(name="wpool", bufs=1))
        xpool = stack.enter_context(tc.tile_pool(name="xpool", bufs=2))
        rawp = stack.enter_context(tc.tile_pool(name="rawp", bufs=2))
        sbig = stack.enter_context(tc.tile_pool(name="sbig", bufs=2))
        stiny = stack.enter_context(tc.tile_pool(name="stiny", bufs=2))
        stiny1 = stack.enter_context(tc.tile_pool(name="stiny1", bufs=1))
        sbig1 = stack.enter_context(tc.tile_pool(name="sbig1", bufs=1))
        stiny3 = stack.enter_context(tc.tile_pool(name="stiny3", bufs=3))
        tpool = stack.enter_context(tc.tile_pool(name="tpool", bufs=4))
        statep = stack.enter_context(tc.tile_pool(name="statep", bufs=1))
        pproj = stack.enter_context(tc.tile_pool(name="pproj", bufs=2, space="PSUM"))
        pddt = stack.enter_context(tc.tile_pool(name="pddt", bufs=2, space="PSUM"))
        pnorm = stack.enter_context(tc.tile_pool(name="pnorm", bufs=2, space="PSUM"))
        psmall = stack.enter_context(tc.tile_pool(name="psmall", bufs=2, space="PSUM"))

        # load constants to SBUF
        ut_sb = consts.tile([L, L], BF16)
        nc.gpsimd.dma_start(ut_sb[:], UT[:])
        sta_sb = consts.tile([L, L], BF16)
        nc.gpsimd.dma_start(sta_sb[:], STA[:])
        onesm_sb = consts.tile([L, L], BF16)
        nc.gpsimd.dma_start(onesm_sb[:], ONESM[:])
        negm_sb = consts.tile([L, L], F32)
        nc.gpsimd.dma_start(negm_sb[:], NEGM[:])
        ident_sb = consts.tile([8, 8], F32)
        nc.gpsimd.dma_start(ident_sb[:], IDENT[:])
        onescol_sb = consts.tile([128, 1], BF16)
        nc.gpsimd.dma_start(onescol_sb[:], ONES_COL[:])
        onesrow_sb = consts.tile([1, 128], F32R)
        nc.gpsimd.dma_start(onesrow_sb[:], ONES_ROW[:].bitcast(F32R))
        convw_sb = consts.tile([128, n_convt, KCONV], F32)
        nc.gpsimd.dma_start(convw_sb[:], convw[:])
        dtb_sb = consts.tile([VH, 1], F32)
        nc.gpsimd.dma_start(dtb_sb[:], dtb[:])
        nega_sb = consts.tile([VH, 1], F32)
        nc.gpsimd.dma_start(nega_sb[:], nega[:])
        normw_sb = consts.tile([128, 1], F32)
        nc.gpsimd.dma_start(normw_sb[:], normw[:])
        eps_sb = consts.tile([1, 1], F32)
        nc.gpsimd.dma_start(eps_sb[:], EPS_T[:])
        onevh_sb = consts.tile([VH, 1], F32)
        nc.gpsimd.dma_start(onevh_sb[:], ONE_VH[:])

        # resident weights
        wt_sb = wpool.tile([128, N_CT, TOTCH], BF16)
        nc.gpsimd.dma_start(wt_sb[:], wt.rearrange("(ko p) f -> p ko f", p=128))
        wout_sb = wpool.tile([128, VH, C], BF16)
        nc.gpsimd.dma_start(wout_sb[:], wout.rearrange("(vo p) f -> p vo f", p=128))

        blocks = [(b, tb) for b in range(B) for tb in range(NTB)]
        xts = {}       # j -> xt tile
        raws = {}      # j -> conv-input raw tile
        zraws = {}     # j -> z pre-activation tile
        accs = {}      # j -> conv accumulator tile [128, n_convt, TB]
        zs = {}        # j -> silu(z) tile
        fqs = {}       # j -> silu'd conv q/k tiles (list)
        vs = {}        # j -> v tile
        qns = {}       # j -> normalized q
        kns = {}       # j -> normalized k
        gsps = {}      # j -> g / softplus(-b) tile
        ogs = {}       # j -> recurrence output
        ktTs = {}      # (j, c) -> transposed k
        vTs = {}       # (j, c) -> transposed v
        Ss = {}        # b -> state tile

        def emit_load(j):
            if j >= NBLK:
                return
            b, tb = blocks[j]
            tsl = slice(tb * TB, (tb + 1) * TB)
            xt_sb = xpool.tile([128, N_CT, TB], BF16, tag="xt")
            nc.gpsimd.dma_start(
                xt_sb[:],
                xt[b].rearrange("(ko p) t -> p ko t", p=128)[:, :, tsl])
            xts[j] = xt_sb

        # ---------------- projection pieces for block j ----------------
        def emit_zproj(j, zi, eng):
            """One z tile: 16 accum MMs + copy to zraw on engine `eng`."""
            if j >= NBLK:
                return
            if zi == 0:
                zraw = sbig.tile([128, n_zt, TB], BF16, tag="zraw")
                zraws[j] = zraw
            xt_sb = xts[j]
            cht = n_convt + zi
            ps = pproj.tile([128, TB], F32, tag="proj")
            wcols = slice(cht * 128, (cht + 1) * 128)
            for ct in range(N_CT):
                nc.tensor.matmul(ps[:], wt_sb[:, ct, wcols], xt_sb[:, ct, :],
                                 start=(ct == 0), stop=(ct == N_CT - 1))
            eng(zraws[j][:, zi, :], ps[:])

        def emit_baproj(j):
            """b/a tile MMs + g / softplus(-b) computed straight from PSUM
            with Exp/Ln (no sigmoid table)."""
            if j >= NBLK:
                return
            xt_sb = xts[j]
            ps = pproj.tile([128, TB], F32, tag="proj")
            rows = slice(0, 32 + VH)
            wcols = slice(n_wt * 128, TOTCH)
            for ct in range(N_CT):
                nc.tensor.matmul(ps[rows, :], wt_sb[:, ct, wcols],
                                 xt_sb[:, ct, :],
                                 start=(ct == 0), stop=(ct == N_CT - 1))
            gsp = stiny.tile([VH, 2, TB], F32, tag="gsp")
            gsps[j] = gsp
            # softplus(-b) = -log(sigmoid(b)) = ln(1 + exp(-b))
            t1 = stiny1.tile([VH, TB], F32, tag="t1")
            nc.scalar.activation(t1[:], ps[0:VH, :], AF.Exp, scale=-1.0)
            # softplus(a + dtb) = ln(1 + exp(a + dtb))
            t2 = stiny1.tile([VH, TB], F32, tag="t2")
            nc.scalar.activation(t2[:], ps[32:32 + VH, :], AF.Exp,
                                 bias=dtb_sb[:])
            nc.scalar.activation(gsp[:, 1, :], t1[:], AF.Ln, bias=onevh_sb[:])
            sp = stiny1.tile([VH, TB], F32, tag="t1")
            nc.scalar.activation(sp[:], t2[:], AF.Ln, bias=onevh_sb[:])
            # g = -exp(A) * softplus(a + dtb)
            nc.vector.tensor_scalar(gsp[:, 0, :], sp[:], nega_sb[:],
                                    None, OP.mult)

        def emit_convproj(j, cht, eng):
            """One conv-channel tile: 16 accum MMs + copy into raw."""
            if j >= NBLK:
                return
            b, tb = blocks[j]
            if cht == 0:
                raw = rawp.tile([128, n_convt, TB + KCONV - 1], BF16, tag="raw")
                raws[j] = raw
                if tb == 0:
                    nc.gpsimd.dma_start(raw[:, :, 0:3], halo[b])
                else:
                    nc.scalar.copy(raw[:, :, 0:3], raws[j - 1][:, :, TB:TB + 3])
                acc = sbig.tile([128, n_convt, TB], BF16, tag="acc", bufs=1)
                accs[j] = acc
            xt_sb = xts[j]
            ps = pproj.tile([128, TB], F32, tag="proj")
            wcols = slice(cht * 128, (cht + 1) * 128)
            for ct in range(N_CT):
                nc.tensor.matmul(ps[:], wt_sb[:, ct, wcols], xt_sb[:, ct, :],
                                 start=(ct == 0), stop=(ct == N_CT - 1))
            eng(raws[j][:, cht, 3:TB + 3], ps[:])

        def emit_conv(j, cht, on_gpsimd=False):
            """4-tap depthwise conv for one channel tile (DVE or GpSimd)."""
            if j >= NBLK:
                return
            raw = raws[j]
            acc = accs[j]
            eng = nc.vector
            eng.tensor_scalar(acc[:, cht, :], raw[:, cht, 0:TB],
                              convw_sb[:, cht, 0:1], None, OP.mult)
            for jj in range(1, KCONV):
                eng.scalar_tensor_tensor(
                    acc[:, cht, :], raw[:, cht, jj:TB + jj],
                    convw_sb[:, cht, jj:jj + 1], acc[:, cht, :],
                    OP.mult, OP.add)

        # ---------------- silu batch for block j ----------------
        def emit_silu(j):
            acc = accs[j]
            z_sb = sbig1.tile([128, n_zt, TB], BF16, tag="z")
            zs[j] = z_sb
            v_sb = sbig1.tile([128, VH, TB], BF16, tag="v")
            vs[j] = v_sb
            fq = []
            for cht in range(2 * KH):
                f = stiny1.tile([128, TB], BF16, tag=f"qkf{cht}")
                nc.scalar.activation(f[:], acc[:, cht, :], AF.Silu)
                fq.append(f)
            fqs[j] = fq
            for cht in range(2 * KH, n_convt):
                nc.scalar.activation(v_sb[:, cht - 2 * KH, :],
                                     acc[:, cht, :], AF.Silu)
            for zi in range(n_zt):
                nc.scalar.activation(z_sb[:, zi, :], zraws[j][:, zi, :],
                                     AF.Silu)
            for c in range(NCH):
                vT = tpool.tile([128, VH, L], BF16, tag="vT")
                for h in range(VH):
                    nc.sync.dma_start_transpose(
                        vT[:, h, :], v_sb[:, h, c * L:c * L + L])
                vTs[(j, c)] = vT

        # ---------------- qk norm for block j (batched, Ln/Exp set) -----
        def emit_norm_pair(srcs, dsts, wedge=None, eps=False):
            """L2-norm two [128,TB] tiles: sq (DVE), ssq (PE), Ln+Exp (ACT),
            rb (PE), mul (DVE) -> dsts. Pairwise keeps pool rotation
            monotone (no FIFO inversion deadlocks). `wedge` emits filler PE
            work between ssq and rb so the PE never stalls on the Ln/Exp."""
            sqs = []
            for f in srcs:
                sq = stiny.tile([128, TB], BF16, tag="sq")
                fa = f[:] if hasattr(f, 'tag') else f
                nc.vector.tensor_tensor(sq[:], fa, fa, OP.mult)
                sqs.append(sq)
            ssqs = []
            for sq in sqs:
                ssq = pnorm.tile([1, TB], F32, tag="nrm")
                nc.tensor.matmul(ssq[:], onescol_sb[:], sq[:],
                                 start=True, stop=True)
                ssqs.append(ssq)
            rinvs = []
            for ssq in ssqs:
                lssq = stiny1.tile([1, TB], F32, tag="lssq", bufs=2)
                if eps:
                    nc.scalar.activation(lssq[:], ssq[:], AF.Ln,
                                         bias=eps_sb[:], scale=1.0 / DV)
                else:
                    nc.scalar.activation(lssq[:], ssq[:], AF.Ln)
                rinv = stiny1.tile([1, TB], F32R, tag="sroot", bufs=2)
                nc.scalar.activation(rinv[:], lssq[:], AF.Exp, scale=-0.5)
                rinvs.append(rinv)
            if wedge is not None:
                wedge()
            for (f, dst, op), rinv in zip(dsts, rinvs):
                rb = pnorm.tile([128, TB], F32, tag="nrm")
                nc.tensor.matmul(rb[:], onesrow_sb[:], rinv[:],
                                 start=True, stop=True)
                if op is None:
                    nc.vector.tensor_tensor(dst, f[:], rb[:], OP.mult)
                else:
                    nc.vector.scalar_tensor_tensor(dst, f[:], normw_sb[:],
                                                   rb[:], OP.mult, OP.mult)

        def emit_qk_transposes(j):
            kn_sb = kns[j]
            for c in range(NCH):
                ktT = tpool.tile([128, KH, L], BF16, tag="ktT")
                for kh in range(KH):
                    nc.sync.dma_start_transpose(
                        ktT[:, kh, :], kn_sb[:, kh, c * L:c * L + L])
                ktTs[(j, c)] = ktT

        # ---------------- one recurrence chunk of block j ----------------
        def emit_chunk(j, c):
            b, tb = blocks[j]
            S = Ss[b]
            gsp = gsps[j]
            qn_sb, kn_sb = qns[j], kns[j]
            z_sb = zs[j]
            og_sb = ogs[j]
            t0 = c * L
            ktT, vT = ktTs[(j, c)], vTs[(j, c)]

            # transpose gsp chunk -> gspT [128, 2VH]
            tps = psmall.tile([128, L], F32, tag="mm128")
            nc.tensor.transpose(tps[:, 0:VH],
                                gsp[:, 0, t0:t0 + L], ident_sb[0:VH, 0:VH])
            nc.tensor.transpose(tps[:, VH:2 * VH],
                                gsp[:, 1, t0:t0 + L], ident_sb[0:VH, 0:VH])
            gspT = stiny3.tile([128, 2 * VH], F32, tag="gspT")
            nc.vector.tensor_copy(gspT[:], tps[:, 0:2 * VH])

            Ball = stiny.tile([128, VH, L], BF16, tag="Ball")
            for h in range(VH):
                nc.vector.tensor_scalar(
                    Ball[:, h, :], ut_sb[:], gspT[:, h:h + 1],
                    None, OP.mult)
            Dps = pddt.tile([128, VH * L], F32, tag="ddt")
            nc.tensor.matmul(Dps[:], sta_sb[:],
                             Ball[:].rearrange("p a b -> p (a b)"),
                             start=True, stop=True)
            dtps = pddt.tile([128, VH * L], F32, tag="ddt")
            nc.tensor.matmul(dtps[:], onesm_sb[:],
                             Ball[:].rearrange("p a b -> p (a b)"),
                             start=True, stop=True)
            ebr = stiny.tile([128, VH, L], F32, tag="ebr")
            nc.scalar.activation(
                ebr[:].rearrange("p a b -> p (a b)"), dtps[:], AF.Exp)
            Eall = stiny.tile([128, VH, L], F32, tag="Eall")
            for h in range(VH):
                # gspT[:, VH+h] holds softplus(-b) = -ln(beta): subtract it
                nc.vector.scalar_tensor_tensor(
                    Eall[:, h, :], Dps[:, h * L:(h + 1) * L],
                    gspT[:, VH + h:VH + h + 1], negm_sb[:],
                    OP.subtract, OP.add)
            Decay = stiny1.tile([128, VH, L], F32, tag="Decay")
            nc.scalar.activation(
                Decay[:].rearrange("p a b -> p (a b)"),
                Eall[:].rearrange("p a b -> p (a b)"), AF.Exp)

            Pps = []
            for kh in range(KH):
                pp = psmall.tile([128, L], F32, tag="mm128")
                nc.tensor.matmul(pp[:], kn_sb[:, kh, t0:t0 + L],
                                 qn_sb[:, kh, t0:t0 + L],
                                 start=True, stop=True)
                Pps.append(pp)

            for h in range(VH):
                kh = h // 2
                qh = stiny3.tile([128, L], F32R, tag="qh")
                nc.vector.tensor_tensor(
                    qh[:], qn_sb[:, kh, t0:t0 + L], ebr[:, h, :],
                    OP.mult)
                PT = stiny3.tile([128, L], BF16, tag="PT")
                nc.vector.tensor_tensor(PT[:], Pps[kh][:],
                                        Decay[:, h, :], OP.mult)
                ops = psmall.tile([128, L], F32, tag="mm128")
                nc.tensor.matmul(ops[:], S[:, h, :], qh[:],
                                 start=True, stop=False)
                nc.tensor.matmul(ops[:], vT[:, h, :], PT[:],
                                 start=False, stop=True)
                nc.vector.tensor_tensor(og_sb[:, h, t0:t0 + L],
                                        ops[:], z_sb[:, h, t0:t0 + L],
                                        OP.mult)
                kt2 = stiny3.tile([128, L], BF16, tag="kt2")
                nc.vector.tensor_scalar(
                    kt2[:], ktT[:, kh, :], Decay[:, h, L - 1:L],
                    None, OP.mult)
                sps = psmall.tile([128, L], F32, tag="mm128")
                nc.tensor.matmul(sps[:], kt2[:], vT[:, h, :],
                                 start=True, stop=True)
                nc.vector.scalar_tensor_tensor(
                    S[:, h, :], S[:, h, :], ebr[:, h, L - 1:L],
                    sps[:], OP.mult, OP.add)

        # ---------------- gated RMSNorm (batched) + out-proj ----------
        def emit_rms_pair(j, ogn_sb, hs, wedge=None):
            og_sb = ogs[j]
            emit_norm_pair([og_sb[:, h, :] for h in hs],
                           [(og_sb[:, h, :], ogn_sb[:, h, :], 'stt')
                            for h in hs], wedge=wedge, eps=True)

        def emit_outproj_tile(j, ogn_sb, c, co, eng):
            """One [L,512] out-proj psum tile + store."""
            b, tb = blocks[j]
            rows = slice(tb * TB + c * L, tb * TB + (c + 1) * L)
            ops2 = pproj.tile([128, 512], F32, tag="proj")
            for h in range(VH):
                nc.tensor.matmul(
                    ops2[:], ogn_sb[:, h, c * L:(c + 1) * L],
                    wout_sb[:, h, co * 512:(co + 1) * 512],
                    start=(h == 0), stop=(h == VH - 1))
            ost = stiny1.tile([128, 512], BF16, tag=f"ost{(c * 4 + co) % 3}")
            eng(ost[:], ops2[:])
            nc.gpsimd.dma_start(out[b, rows, co * 512:(co + 1) * 512], ost[:])

        # engines for copies, round-robin
        def cp_scalar(dst, src):
            nc.scalar.copy(dst, src)

        def cp_vector(dst, src):
            nc.vector.tensor_copy(dst, src)

        def cp_gpsimd(dst, src):
            nc.gpsimd.tensor_copy(dst, src)

        # ================= pipelined emission =================
        # prologue: block 0's projections + conv, block 1's loads
        emit_load(0)
        emit_load(1)
        for zi in range(n_zt):
            emit_zproj(0, zi, cp_vector)
        emit_baproj(0)
        for cht in range(n_convt):
            emit_convproj(0, cht, cp_scalar)
        for cht in range(n_convt):
            emit_conv(0, cht)

        ogn_prev = None
        for j in range(NBLK):
            b, tb = blocks[j]
            emit_load(j + 2)

            # ---- phase A: rms(j-1) + outproj(j-1) + z/ba proj(j+1) ----
            emit_zproj(j + 1, 0, cp_vector)
            if j >= 1:
                ogn_prev = sbig1.tile([128, VH, TB], BF16, tag="ogn")
                emit_rms_pair(j - 1, ogn_prev, [0, 1],
                              wedge=lambda: emit_zproj(j + 1, 1, cp_vector))
                emit_rms_pair(j - 1, ogn_prev, [2, 3],
                              wedge=lambda: emit_zproj(j + 1, 2, cp_vector))
            else:
                emit_zproj(j + 1, 1, cp_vector)
                emit_zproj(j + 1, 2, cp_vector)
            if j >= 1:
                for c in range(NCH):
                    for co in range(2):
                        emit_outproj_tile(j - 1, ogn_prev, c, co,
                                          cp_scalar if co == 0 else cp_vector)
            emit_zproj(j + 1, 3, cp_vector)
            emit_baproj(j + 1)
            if j >= 1:
                for c in range(NCH):
                    emit_outproj_tile(j - 1, ogn_prev, c, 2,
                                      cp_scalar if c < 2 else cp_vector)
            for cht in range(4):
                emit_convproj(j + 1, cht, cp_scalar)
                emit_conv(j + 1, cht)

            # ---- phase B: silu batch for block j ----
            emit_silu(j)

            # ---- phase C: qknorm(j) + chunks(j), wedged with proj(j+1) --
            fq = fqs[j]
            qn_sb = sbig1.tile([128, KH, TB], BF16, tag="qn")
            kn_sb = sbig1.tile([128, KH, TB], BF16, tag="kn")
            qns[j], kns[j] = qn_sb, kn_sb
            emit_norm_pair(fq[0:2],
                           [(fq[0], qn_sb[:, 0, :], None),
                            (fq[1], qn_sb[:, 1, :], None)],
                           wedge=lambda: (emit_convproj(j + 1, 4, cp_vector),
                                          emit_conv(j + 1, 4)))
            emit_norm_pair(fq[2:4],
                           [(fq[2], kn_sb[:, 0, :], None),
                            (fq[3], kn_sb[:, 1, :], None)],
                           wedge=lambda: (emit_convproj(j + 1, 5, cp_vector),
                                          emit_conv(j + 1, 5)))
            emit_qk_transposes(j)

            if tb == 0:
                S = statep.tile([128, VH, DV], F32R, tag=f"S{b}")
                nc.gpsimd.dma_start(
                    S[:], s0[b].rearrange("h d v -> d h v").bitcast(F32R))
                Ss[b] = S
            og_sb = sbig.tile([128, VH, TB], BF16, tag="og", bufs=1)
            ogs[j] = og_sb

            for c in range(NCH):
                if c == 1:
                    emit_convproj(j + 1, 6, cp_vector)
                    emit_conv(j + 1, 6)
                if c == 2:
                    emit_convproj(j + 1, 7, cp_vector)
                    emit_conv(j + 1, 7)
                if c >= 1 and j >= 1:
                    # out-proj co=3 tiles as PE fillers inside the chunk loop
                    emit_outproj_tile(j - 1, ogn_prev, c - 1, 3, cp_vector)
                emit_chunk(j, c)
            if j >= 1:
                emit_outproj_tile(j - 1, ogn_prev, 3, 3, cp_vector)

        # epilogue
        ogn_last = sbig1.tile([128, VH, TB], BF16, tag="ogn")
        emit_rms_pair(NBLK - 1, ogn_last, [0, 1])
        emit_rms_pair(NBLK - 1, ogn_last, [2, 3])
        for c in range(NCH):
            for co in range(4):
                emit_outproj_tile(NBLK - 1, ogn_last, c, co,
                                  (cp_scalar, cp_vector, cp_scalar,
                                   cp_vector)[co])

    return nc


_NC_CACHE = None
LAST_RESULT = None


def kernel(**inputs):
    global _NC_CACHE, LAST_RESULT
    x = np.asarray(inputs["x"], np.float32)
    input_pos = np.asarray(inputs["input_pos"])
    W_qkv = np.asarray(inputs["W_qkv"], np.float32)
    W_z = np.asarray(inputs["W_z"], np.float32)
    W_b = np.asarray(inputs["W_b"], np.float32)
    W_a = np.asarray(inputs["W_a"], np.float32)
    conv_w = np.asarray(inputs["conv_w"], np.float32)[:, 0, :]
    dt_bias = np.asarray(inputs["dt_bias"], np.float32)
    A_log = np.asarray(inputs["A_log"], np.float32)
    norm_w = np.asarray(inputs["norm_w"], np.float32)
    W_out = np.asarray(inputs["W_out"], np.float32)
    conv_state = np.asarray(inputs["conv_state"], np.float32)
    rec_state = np.asarray(inputs["recurrent_state"], np.float32)

    keep = 0.0 if int(input_pos[0]) == 0 else 1.0
    conv_state = conv_state * keep
    rec_state = rec_state * keep

    xt_host = np.ascontiguousarray(x.transpose(0, 2, 1)).astype(BF16_NP)

    in_maps = []
    for core in range(NCORES):
        vh = slice(VH * core, VH * (core + 1))
        qrows = slice(QCH * core, QCH * (core + 1))
        krows = slice(KEY + QCH * core, KEY + QCH * (core + 1))
        vrows = slice(2 * KEY + VCH * core, 2 * KEY + VCH * (core + 1))
        zrows = slice(ZCH * core, ZCH * (core + 1))

        Wt = np.concatenate(
            [W_qkv[qrows], W_qkv[krows], W_qkv[vrows], W_z[zrows],
             W_b[vh], np.zeros((32 - VH, C), np.float32),
             W_a[vh]], axis=0)                    # [TOTCH, C]
        wt_host = np.ascontiguousarray(Wt.T).astype(BF16_NP)      # [C, TOTCH]
        wout_host = np.ascontiguousarray(
            W_out[:, VCH * core:VCH * (core + 1)].T).astype(BF16_NP)

        cw = np.concatenate([conv_w[qrows], conv_w[krows], conv_w[vrows]], 0)
        convw_host = np.ascontiguousarray(
            cw.reshape(CONVCH // 128, 128, KCONV).transpose(1, 0, 2))

        cs = np.concatenate([conv_state[:, qrows], conv_state[:, krows],
                             conv_state[:, vrows]], axis=1)       # [B,1024,4]
        halo_host = np.ascontiguousarray(
            cs[:, :, 1:4].reshape(B, CONVCH // 128, 128, 3)
            .transpose(0, 2, 1, 3)).astype(BF16_NP)

        s0_host = np.ascontiguousarray(rec_state[:, vh])          # [B,VH,DK,DV]
        dtb_host = np.ascontiguousarray(dt_bias[vh][:, None])
        nega_host = np.ascontiguousarray(-np.exp(A_log[vh])[:, None])
        normw_host = np.ascontiguousarray(norm_w[:, None])

        in_maps.append({
            "xt": xt_host, "wt": wt_host, "wout": wout_host,
            "convw": convw_host, "halo": halo_host, "s0": s0_host,
            "dtb": dtb_host, "nega": nega_host, "normw": normw_host,
        })

    if _NC_CACHE is None:
        _NC_CACHE = build_kernel()
    res = run_bass_kernel_spmd(_NC_CACHE, in_maps, core_ids=list(range(NCORES)))
    LAST_RESULT = res

    total = np.zeros((B, T, C), np.float32)
    for r in res.results:
        total += np.asarray(r["out"], dtype=np.float32)
    return total
